# revision 45
# baseline (speedup 1.0000x reference)
"""Trainium2 Bass kernel for nn_BDHNet_35905926595181.

Strategy
--------
The T=256 Hebbian plasticity scan has a closed form (decay-masked linear
attention): with per-step coefficients a_hat/e_hat derived from (mask, alpha,
eta),

    y[b,t,h,:] = C0[b,h,t] * (adj[b] @ x[b,t]) + sum_{s<t} L[b,h,t,s] * (x_t . x_s) * x[b,s]
    w_final[b,h] = C0[b,h,T] * adj[b] + X^T diag(d[b,h]) X

so the whole scan becomes matmuls (Gram matrix G = X X^T + masked attention).

Sharding (one SPMD launch on 8 cores, no collectives):
  Part 1 (batch-parallel): core q computes pooling vals/scores and the
    trajectory head for samples {2q, 2q+1}, accumulating over all 8 heads
    directly in PSUM.
  Part 2 (head-parallel): core q computes w_final[:, q] for all 16 samples and
    the big classifier matmul fused(:, head-q block) @ W1[block] with W1
    reduction-sharded (11.3 MB/core streamed, instead of 90 MB replicated).
Host does only tiny glue: coefficient precompute, softmax/context, classifier
tail (16x256 adds, 16x256x2 matmul), output assembly.
"""

import os
import numpy as np

import concourse.bass as bass
import concourse.mybir as mybir
import concourse.tile as tile
from concourse import bacc
import concourse.bass_utils as bass_utils

B, T, N, H, P, C = 16, 256, 105, 8, 64, 2
NC = 8
CH = 128                 # partition chunk of T
NK = T // CH             # 2 chunks
FLATH = N * N            # 11025 rows of W1 per head
NGRP = 7                 # W1 streamed in 7 groups of 15 n-rows
GN = FLATH // NGRP // N  # 15 n's per group
F32 = mybir.dt.float32
F32R = mybir.dt.float32r
F16 = mybir.dt.float16

# packed small-input layouts: name -> (row_count, col_offset, width).
# pack0 carries the chain-critical sections (Gram lhsT, exp-mask inputs) so
# the part-1 producer pipeline starts ~4us earlier than with one big pack.
PACK0_SECT = {"xbT": (105, 0, 512), "dmat": (128, 512, 512),
              "lnab": (128, 1024, 16)}
PACK0_W = 1040
PACKR_SECT = {"xp": (128, 0, 420), "adjTp": (105, 420, 210),
              "wvu": (105, 630, 520), "wtrajT": (105, 1150, 105)}
PACKR_W = 1255
PACKF_SECT = {"dsq": (128, 0, 32), "ptv": (105, 32, 16),
              "ident": (105, 48, 105), "adjTg": (105, 153, 1680)}
PACKF_W = 1833

_progs = {}
last_exec_time_ns = None
last_results = None


def _build(n_lb, dev_lt):
    """Build the SPMD program.
    n_lb=1: L/C0 shared across the pair (uniform mask); n_lb=2: per-sample.
    dev_lt: build the decay masks L (and C0) on device via ACT exp from a
    delta matrix instead of DMAing ~2.9 MB of host-precomputed masks."""
    nc = bacc.Bacc(None, target_bir_lowering=False)

    # Small inputs packed into two tensors (one DMA each): per-DMA issue
    # overhead (~0.65us) otherwise idles the DMA engines early on.
    pack0 = nc.dram_tensor("pack0", [CH, PACK0_W], F32R, kind="ExternalInput")
    packr = nc.dram_tensor("packr", [CH, PACKR_W], F32R, kind="ExternalInput")
    packf = nc.dram_tensor("packf", [CH, PACKF_W], F32, kind="ExternalInput")
    if not dev_lt:
        lt = nc.dram_tensor("lt", [CH, n_lb * H * 2 * T], F32, kind="ExternalInput")
        c0m = nc.dram_tensor("c0m", [N, n_lb * H * T], F32, kind="ExternalInput")
    xall = nc.dram_tensor("xall", [CH, B * NK * N], F32, kind="ExternalInput")
    # W1 head-slice, host-prearranged to [m, n*256 + j] so every DMA reads
    # long contiguous runs (full DMA rate)
    w1s = nc.dram_tensor("w1s", [N, (FLATH // N) * 256], F16, kind="ExternalInput")

    wf_out = nc.dram_tensor("wf_out", [N, B * N], F32, kind="ExternalOutput")
    cls_out = nc.dram_tensor("cls_out", [B, 256], F32, kind="ExternalOutput")
    vs_out = nc.dram_tensor("vs_out", [2, P + 1, T], F32, kind="ExternalOutput")
    z_out = nc.dram_tensor("z_out", [2, N, T], F32, kind="ExternalOutput")

    with tile.TileContext(nc) as tc:
        with (
            tc.tile_pool(name="consts", bufs=1) as consts,
            tc.tile_pool(name="work", bufs=2) as work,
            tc.tile_pool(name="spool", bufs=3) as spool,
            tc.tile_pool(name="ypool", bufs=1) as ypool,
            tc.tile_pool(name="w1pool", bufs=1) as w1pool,
            tc.tile_pool(name="psG", bufs=1, space="PSUM") as psG,
            tc.tile_pool(name="psY", bufs=2, space="PSUM") as psY,
            tc.tile_pool(name="psV", bufs=1, space="PSUM") as psV,
            tc.tile_pool(name="psZ", bufs=1, space="PSUM") as psZ,
            tc.tile_pool(name="psWF", bufs=2, space="PSUM") as psWF,
            tc.tile_pool(name="psCLS", bufs=1, space="PSUM") as psCLS,
        ):
            # ---- load constants (ordered by consumer urgency; SP queue is
            # in-order, so nothing compute-dependent may sit between these) ----
            cts = {}
            pack0_t = consts.tile([CH, PACK0_W], F32R, name="pack0_t")
            nc.sync.dma_start(out=pack0_t[:], in_=pack0[:])
            packr_t = consts.tile([CH, PACKR_W], F32R, name="packr_t")
            nc.sync.dma_start(out=packr_t[:], in_=packr[:])
            packf_t = consts.tile([CH, PACKF_W], F32, name="packf_t")
            nc.sync.dma_start(out=packf_t[:], in_=packf[:])
            for name, (rows, off, wid) in PACK0_SECT.items():
                ap = pack0_t[0:rows, off:off + wid]
                cts[name] = ap.bitcast(F32) if name in ("dmat", "lnab") else ap
            for name, (rows, off, wid) in PACKR_SECT.items():
                cts[name] = packr_t[0:rows, off:off + wid]
            for name, (rows, off, wid) in PACKF_SECT.items():
                cts[name] = packf_t[0:rows, off:off + wid]
            if not dev_lt:
                for name, hdl in [("lt", lt), ("c0m", c0m)]:
                    t = consts.tile(list(hdl.shape), F32, name=f"c_{name}")
                    nc.sync.dma_start(out=t[:], in_=hdl[:])
                    cts[name] = t
            xall_t = consts.tile([CH, B * NK * N], F32, name="xall_t")
            nc.sync.dma_start(out=xall_t[:], in_=xall[:])
            cts["xall"] = xall_t[:]
            # preload the whole W1 slice (7 groups, no slot reuse) so the SP
            # queue never stalls behind a waiting DMA
            w1tiles = []
            for g in range(NGRP):
                w1t = w1pool.tile([N, GN * 256], F16, name=f"w1t{g}", tag=f"w1t{g}")
                nc.sync.dma_start(out=w1t[:],
                                  in_=w1s[:, g * GN * 256:(g + 1) * GN * 256])
                w1tiles.append(w1t)
            wfT_r = consts.tile([N, B * N], F16, name="wfT_r")    # [m, n*16+b]
            wfT_c = consts.tile([N, B * N], F32, name="wfT_c")    # [m, b*105+n]

            # Gram matmuls first: they only need xbT (arrives first) and
            # unblock the whole part-1 producer pipeline.
            part1 = {}
            emitted_g = []

            def emit_g(i):
                g_ps = psG.tile([CH, 2 * T], F32, name="g_ps")
                for k in range(NK):
                    nc.tensor.matmul(
                        g_ps[:, k * T:(k + 1) * T],
                        cts["xbT"][:, i * T + k * CH: i * T + (k + 1) * CH],
                        cts["xbT"][:, i * T:(i + 1) * T],
                        start=True, stop=True)
                g_sb = work.tile([CH, 2 * T], F32, name="g_sb")
                nc.scalar.copy(out=g_sb[:], in_=g_ps[:])
                part1[i] = dict(g_sb=g_sb)

            emit_g(0)
            emit_g(1)

            # ---- part 2 (emitted in chunks interleaved with part 1):
            # w_final for all 16 samples for this core's head. Elementwise on
            # DVE; wfT copies on the otherwise-idle GpSimd engine.
            def emit_p2(b):
                xck = work.tile([CH, NK * N], F32, name="xck")
                for k in range(NK):
                    nc.vector.tensor_scalar_mul(
                        xck[:, k * N:(k + 1) * N],
                        cts["xall"][:, (b * NK + k) * N:(b * NK + k + 1) * N],
                        cts["dsq"][:, b * NK + k: b * NK + k + 1])
                adjs = work.tile([N, N], F32, name="adjs")
                nc.vector.tensor_scalar_mul(
                    adjs[:], cts["adjTg"][:, b * N:(b + 1) * N],
                    cts["ptv"][:, b:b + 1])
                wf_ps = psWF.tile([N, N], F32, name="wf_ps")
                for k in range(NK):
                    nc.tensor.matmul(
                        wf_ps[:], xck[:, k * N:(k + 1) * N],
                        xck[:, k * N:(k + 1) * N], start=(k == 0), stop=False)
                nc.tensor.matmul(
                    wf_ps[:], cts["ident"][:], adjs[:], start=False, stop=True)
                nc.scalar.copy(out=wfT_r[:, b:B * N:B], in_=wf_ps[:])
                nc.vector.tensor_copy(out=wfT_c[:, b * N:(b + 1) * N], in_=wf_ps[:])

            if dev_lt:
                # L_h[s, k*256+t] = eta*alpha^(t-1-s) masked: exp(lnA*dmat+lnE);
                # dmat holds +1e4 where t<=s so exp underflows to exactly 0.
                # Emitted after part 2 so these ACT ops don't block its copies.
                lm_tiles, c0_tiles = [], []
                trow = consts.tile([N, T], F32, name="trow")
                nc.gpsimd.iota(trow[:], pattern=[[1, T]], base=0,
                               channel_multiplier=0,
                               allow_small_or_imprecise_dtypes=True)
                for h in range(H):
                    lmt = consts.tile([CH, 2 * T], F32, name=f"lm{h}")
                    nc.scalar.activation(
                        out=lmt[:], in_=cts["dmat"][:],
                        func=mybir.ActivationFunctionType.Exp,
                        bias=cts["lnab"][:, 2 * h + 1:2 * h + 2],
                        scale=cts["lnab"][:, 2 * h:2 * h + 1])
                    lm_tiles.append(lmt)
                    c0t = consts.tile([N, T], F32, name=f"c0_{h}")
                    nc.scalar.activation(
                        out=c0t[:], in_=trow[:],
                        func=mybir.ActivationFunctionType.Exp,
                        bias=0.0,
                        scale=cts["lnab"][0:N, 2 * h:2 * h + 1])
                    c0_tiles.append(c0t)

            # ---- interleaved emission: cls groups + part-1 head iterations ----
            cls_ps = psCLS.tile([B, 256], F32, name="cls_ps")

            def emit_cls_group(g):
                w1t = w1tiles[g]
                for ii in range(GN):
                    n = g * GN + ii
                    nc.tensor.matmul(
                        cls_ps[:], wfT_r[:, n * B:(n + 1) * B],
                        w1t[:, ii * 256:(ii + 1) * 256],
                        start=(n == 0), stop=(n == FLATH // N - 1))

            def emit_head(i, h):
                st = part1[i]
                if dev_lt:
                    lt_ap = lm_tiles[h][:]
                    c0_ap = c0_tiles[h][:]
                else:
                    lo = ((i * H + h) if n_lb == 2 else h) * 2 * T
                    lt_ap = cts["lt"][:, lo:lo + 2 * T]
                    co = ((i * H + h) if n_lb == 2 else h) * T
                    c0_ap = cts["c0m"][:, co:co + T]
                s_sb = spool.tile([CH, 2 * T], F32R, name="s_sb")
                nc.vector.tensor_mul(out=s_sb[:], in0=lt_ap, in1=st["g_sb"][:])
                xh = spool.tile([N, T], F32R, name="xh")
                nc.vector.tensor_mul(
                    out=xh[:], in0=cts["xbT"][:, i * T:(i + 1) * T], in1=c0_ap)
                y_ps = psY.tile([N, T], F32, name="y_ps")
                for k in range(NK):
                    nc.tensor.matmul(
                        y_ps[:],
                        cts["xp"][:, (i * NK + k) * N:(i * NK + k + 1) * N],
                        s_sb[:, k * T:(k + 1) * T],
                        start=(k == 0), stop=False)
                nc.tensor.matmul(
                    y_ps[:], cts["adjTp"][:, i * N:(i + 1) * N], xh[:],
                    start=False, stop=True)
                y_sb = ypool.tile([N, T], F32R, name=f"y_sb{i}_{h}")
                nc.scalar.copy(out=y_sb[:], in_=y_ps[:])
                st[h] = y_sb

            def emit_vz(i):
                # batched V/Z accumulation: avoids the per-head PE<->ACT
                # round-trip latency in the chain
                st = part1[i]
                st["v_ps"] = psV.tile([P + 1, T], F32, name="v_ps", tag="v_ps")
                st["z_ps"] = psZ.tile([N, T], F32, name="z_ps", tag="z_ps")
                for h in range(H):
                    nc.tensor.matmul(
                        st["v_ps"][:], cts["wvu"][:, h * (P + 1):(h + 1) * (P + 1)],
                        st[h][:], start=(h == 0), stop=(h == H - 1))
                for h in range(H):
                    nc.tensor.matmul(
                        st["z_ps"][:], cts["wtrajT"][:], st[h][:],
                        start=(h == 0), stop=(h == H - 1))

            def emit_finish(i):
                st = part1[i]
                vs_sb = work.tile([P + 1, T], F32, name="vs_sb")
                nc.scalar.copy(out=vs_sb[:], in_=st["v_ps"][:])
                nc.sync.dma_start(out=vs_out[i], in_=vs_sb[:])
                z_sb = work.tile([N, T], F32, name="z_sb")
                nc.vector.tensor_copy(out=z_sb[:], in_=st["z_ps"][:])
                nc.sync.dma_start(out=z_out[i], in_=z_sb[:])

            # schedule: part-2 chunks and cls groups interleaved with
            # part-1 head iterations (engines execute in emission order)
            plan = [
                ("P2", range(0, 16)),
                ("CLS", 0), ("HD", (0, 0)), ("HD", (0, 1)), ("HD", (0, 2)),
                ("CLS", 1), ("HD", (0, 3)), ("HD", (0, 4)), ("HD", (0, 5)),
                ("CLS", 2), ("HD", (0, 6)), ("HD", (0, 7)), ("HD", (1, 0)),
                ("CLS", 3), ("VZ", 0), ("FIN", 0), ("HD", (1, 1)),
                ("CLS", 4), ("HD", (1, 2)), ("HD", (1, 3)), ("HD", (1, 4)),
                ("CLS", 5), ("HD", (1, 5)), ("HD", (1, 6)), ("HD", (1, 7)),
                ("CLS", 6), ("VZ", 1), ("FIN", 1),
            ]
            for kind, arg in plan:
                if kind == "P2":
                    for b_ in arg:
                        emit_p2(b_)
                elif kind == "HD":
                    emit_head(*arg)
                elif kind == "CLS":
                    emit_cls_group(arg)
                elif kind == "VZ":
                    emit_vz(arg)
                else:
                    emit_finish(arg)
            nc.sync.dma_start(out=wf_out[:], in_=wfT_c[:])
            cls_sb = work.tile([B, 256], F32, name="cls_sb")
            nc.scalar.copy(out=cls_sb[:], in_=cls_ps[:])
            nc.sync.dma_start(out=cls_out[:], in_=cls_sb[:])

    nc.finalize()
    return nc


def _coeffs(mask, alpha, eta):
    """Mask-general scan coefficients in float64 log space.
    C0[b,h,t] = prod_{r<t} a_hat (t=0..T); LT[b,h,s,t] = L[b,h,t,s];
    d[b,h,s] = e_s * prod_{r=s+1..T-1} a_hat."""
    m = mask.astype(np.float64)
    al = np.asarray(alpha, np.float64)
    et = np.asarray(eta, np.float64)
    a_hat = m[:, None, :] * al[None, :, None] + (1.0 - m[:, None, :])
    e_hat = m[:, None, :] * et[None, :, None]
    logc = np.concatenate(
        [np.zeros((B, H, 1)), np.cumsum(np.log(a_hat), axis=2)], axis=2)
    C0 = np.exp(logc)
    t_idx = np.arange(T)
    valid = (t_idx[None, :] > t_idx[:, None])          # [s,t]: s < t
    dlog = logc[:, :, None, 0:T] - logc[:, :, 1:T + 1, None]   # [b,h,s,t]
    with np.errstate(over="ignore"):
        LT = np.where(valid[None, None],
                      np.exp(np.where(valid[None, None], dlog, -np.inf))
                      * e_hat[:, :, :, None], 0.0)
    d = e_hat * np.exp(logc[:, :, T:T + 1] - logc[:, :, 1:T + 1])
    return C0, LT, d


def kernel(x, adj, mask, alpha, eta, query, Wk, bk, Wv, bv, Wtraj, btraj,
           W1, b1, W2, b2):
    global last_exec_time_ns, last_results
    f = np.float32
    x = np.ascontiguousarray(np.asarray(x, f))
    adj = np.ascontiguousarray(np.asarray(adj, f))
    mask = np.asarray(mask)
    alpha = np.asarray(alpha, f); eta = np.asarray(eta, f)
    query = np.asarray(query, f)
    Wk = np.asarray(Wk, f); bk = np.asarray(bk, f)
    Wv = np.asarray(Wv, f); bv = np.asarray(bv, f)
    Wtraj = np.asarray(Wtraj, f); btraj = np.asarray(btraj, f)
    W1 = np.asarray(W1, f); b1 = np.asarray(b1, f)
    W2 = np.asarray(W2, f); b2 = np.asarray(b2, f)

    uniform = bool(np.all(mask == mask[0:1]))
    dev_lt = uniform and bool(np.all(alpha >= 0) and np.all(alpha <= 0.99))
    n_lb = 1 if uniform else 2
    key = (n_lb, dev_lt)
    if key not in _progs:
        _progs[key] = _build(n_lb, dev_lt)
    nc = _progs[key]

    C0, LT, d = _coeffs(mask, alpha, eta)
    C0f = C0.astype(f)
    dsq_all = np.sqrt(d).astype(f)                      # (B,H,T)
    q = query[0, 0]
    u = (Wk @ q).astype(f)                              # (840,)
    wvu_h = np.empty((H, N, P + 1), f)
    wvu_h[:, :, :P] = Wv.reshape(H, N, P)
    wvu_h[:, :, P] = u.reshape(H, N)
    wvu_arr = np.ascontiguousarray(wvu_h.transpose(1, 0, 2).reshape(N, H * (P + 1)))
    wtrajT_arr = np.ascontiguousarray((Wtraj / float(H)).T)
    xall_arr = np.ascontiguousarray(
        x.reshape(B, NK, CH, N).transpose(2, 0, 1, 3).reshape(CH, B * NK * N))
    adjTg_arr = np.ascontiguousarray(adj.transpose(2, 0, 1).reshape(N, B * N))
    ident_arr = np.eye(N, dtype=f)

    if dev_lt:
        # device builds L/C0 via exp(lnA*delta + lnE); +1e4 in the masked
        # region underflows exp to exactly 0
        s_idx = np.arange(T)[:, None]
        t_idx = np.arange(T)[None, :]
        dm = np.where(t_idx > s_idx, (t_idx - 1 - s_idx).astype(np.float64), 1e4)
        dmat_arr = np.ascontiguousarray(
            dm.astype(f).reshape(NK, CH, T).transpose(1, 0, 2).reshape(CH, 2 * T))
        lnab_arr = np.empty((CH, 2 * H), f)
        lnab_arr[:, 0::2] = np.log(np.maximum(alpha, 1e-30))[None, :]
        lnab_arr[:, 1::2] = np.log(np.maximum(eta, 1e-38))[None, :]
    elif uniform:
        lt_shared = np.ascontiguousarray(
            LT[0].astype(f).reshape(H, NK, CH, T).transpose(2, 0, 1, 3)
            .reshape(CH, H * 2 * T))
        c0_shared = np.ascontiguousarray(np.broadcast_to(
            C0f[0, :, :T].reshape(1, H * T), (N, H * T)))

    # W1 head-blocks rearranged to [h][m, n*256+j], cast fp16 (one 90MB pass)
    W1res = np.ascontiguousarray(
        W1[:H * FLATH].reshape(H, N, N, 256).transpose(0, 2, 1, 3)
        .reshape(H, N, (FLATH // N) * 256).astype(np.float16))

    def _put(dst, sect, name, arr):
        rows, off, wid = sect[name]
        assert arr.shape == (rows, wid), (name, arr.shape, (rows, wid))
        dst[:rows, off:off + wid] = arr

    # shared sections of the packs
    pack0_tpl = np.zeros((CH, PACK0_W), f)
    if dev_lt:
        _put(pack0_tpl, PACK0_SECT, "dmat", dmat_arr)
        _put(pack0_tpl, PACK0_SECT, "lnab", lnab_arr)
    packr_tpl = np.zeros((CH, PACKR_W), f)
    _put(packr_tpl, PACKR_SECT, "wvu", wvu_arr)
    _put(packr_tpl, PACKR_SECT, "wtrajT", wtrajT_arr)
    packf_tpl = np.zeros((CH, PACKF_W), f)
    _put(packf_tpl, PACKF_SECT, "ident", ident_arr)
    _put(packf_tpl, PACKF_SECT, "adjTg", adjTg_arr)

    in_maps = []
    for qc in range(NC):
        pair = slice(2 * qc, 2 * qc + 2)
        xpair = x[pair]                                  # (2,T,N)
        pack0_a = pack0_tpl.copy()
        _put(pack0_a, PACK0_SECT, "xbT",
             xpair.transpose(0, 2, 1).transpose(1, 0, 2).reshape(N, 2 * T))
        packr_a = packr_tpl.copy()
        _put(packr_a, PACKR_SECT, "xp", np.ascontiguousarray(
            xpair.reshape(2, NK, CH, N).transpose(2, 0, 1, 3).reshape(CH, 2 * NK * N)))
        _put(packr_a, PACKR_SECT, "adjTp",
             adj[pair].transpose(2, 0, 1).reshape(N, 2 * N))
        packf_a = packf_tpl.copy()
        _put(packf_a, PACKF_SECT, "dsq",
             dsq_all[:, qc, :].reshape(B, NK, CH).transpose(2, 0, 1).reshape(CH, B * NK))
        _put(packf_a, PACKF_SECT, "ptv",
             np.broadcast_to(C0f[:, qc, T].reshape(1, B), (N, B)))
        im = {"pack0": pack0_a, "packr": packr_a, "packf": packf_a,
              "xall": xall_arr, "w1s": W1res[qc]}
        if not dev_lt:
            if uniform:
                im["lt"] = lt_shared
                im["c0m"] = c0_shared
            else:
                im["lt"] = np.ascontiguousarray(
                    LT[pair].astype(f).reshape(2, H, NK, CH, T)
                    .transpose(3, 0, 1, 2, 4).reshape(CH, 2 * H * 2 * T))
                im["c0m"] = np.ascontiguousarray(np.broadcast_to(
                    C0f[pair, :, :T].reshape(1, 2 * H * T), (N, 2 * H * T)))
        in_maps.append(im)

    trace = os.environ.get("BDH_TRACE", "") == "1"
    res = bass_utils.run_bass_kernel_spmd(
        nc, in_maps, list(range(NC)), trace=trace)
    last_exec_time_ns = res.exec_time_ns
    last_results = res

    # ---- host assembly ----
    w_final = np.empty((B, H, N, N), f)
    scores_raw = np.empty((B, T), f)
    VT = np.empty((B, P, T), f)
    x_next = np.empty((B, T, N), f)
    cls_sum = np.zeros((B, 256), np.float64)
    for qc in range(NC):
        r = res.results[qc]
        w_final[:, qc] = r["wf_out"].reshape(N, B, N).transpose(1, 2, 0)
        cls_sum += r["cls_out"].astype(np.float64)
        for i in range(2):
            b = 2 * qc + i
            scores_raw[b] = r["vs_out"][i, P]
            VT[b] = r["vs_out"][i, :P]
            x_next[b] = r["z_out"][i].T + btraj

    scale = 1.0 / np.sqrt(np.float32(P))
    scores = (scores_raw + np.float32(bk @ q)) * scale
    scores = np.where(mask == 0, f(-1e9), scores).astype(f)
    ex = np.exp((scores - scores.max(axis=1, keepdims=True)).astype(np.float64))
    attn = ex / ex.sum(axis=1, keepdims=True)
    context = np.einsum("bt,bpt->bp", attn, VT.astype(np.float64)) + bv

    W1c = W1[H * FLATH:, :].astype(np.float64)           # context rows of W1
    h1 = cls_sum + context @ W1c + b1
    h1 = np.maximum(h1, 0.0)
    logits = (h1 @ W2.astype(np.float64) + b2).astype(f)

    return logits, x_next, w_final


# revision 46
# speedup vs baseline: 1.0069x; 1.0069x over previous
"""Trainium2 Bass kernel for nn_BDHNet_35905926595181.

Strategy
--------
The T=256 Hebbian plasticity scan has a closed form (decay-masked linear
attention): with per-step coefficients a_hat/e_hat derived from (mask, alpha,
eta),

    y[b,t,h,:] = C0[b,h,t] * (adj[b] @ x[b,t]) + sum_{s<t} L[b,h,t,s] * (x_t . x_s) * x[b,s]
    w_final[b,h] = C0[b,h,T] * adj[b] + X^T diag(d[b,h]) X

so the whole scan becomes matmuls (Gram matrix G = X X^T + masked attention).

Sharding (one SPMD launch on 8 cores, no collectives):
  Part 1 (batch-parallel): core q computes pooling vals/scores and the
    trajectory head for samples {2q, 2q+1}, accumulating over all 8 heads
    directly in PSUM.
  Part 2 (head-parallel): core q computes w_final[:, q] for all 16 samples and
    the big classifier matmul fused(:, head-q block) @ W1[block] with W1
    reduction-sharded (11.3 MB/core streamed, instead of 90 MB replicated).
Host does only tiny glue: coefficient precompute, softmax/context, classifier
tail (16x256 adds, 16x256x2 matmul), output assembly.
"""

import os
import numpy as np

import concourse.bass as bass
import concourse.mybir as mybir
import concourse.tile as tile
from concourse import bacc
import concourse.bass_utils as bass_utils

B, T, N, H, P, C = 16, 256, 105, 8, 64, 2
NC = 8
CH = 128                 # partition chunk of T
NK = T // CH             # 2 chunks
FLATH = N * N            # 11025 rows of W1 per head
NGRP = 7                 # W1 streamed in 7 groups of 15 n-rows
GN = FLATH // NGRP // N  # 15 n's per group
F32 = mybir.dt.float32
F32R = mybir.dt.float32r
F16 = mybir.dt.float16

# packed small-input layouts: name -> (row_count, col_offset, width).
# pack0 carries the chain-critical sections (Gram lhsT, exp-mask inputs) so
# the part-1 producer pipeline starts ~4us earlier than with one big pack.
PACK0_SECT = {"xbT": (105, 0, 512), "dmat": (128, 512, 512),
              "lnab": (128, 1024, 16)}
PACK0_W = 1040
PACKR_SECT = {"xp": (128, 0, 420), "adjTp": (105, 420, 210),
              "wvu": (105, 630, 520), "wtrajT": (105, 1150, 105)}
PACKR_W = 1255
PACKF_SECT = {"dsq": (128, 0, 32), "ptv": (105, 32, 16),
              "ident": (105, 48, 105), "adjTg": (105, 153, 1680)}
PACKF_W = 1833

_progs = {}
last_exec_time_ns = None
last_results = None


def _build(n_lb, dev_lt):
    """Build the SPMD program.
    n_lb=1: L/C0 shared across the pair (uniform mask); n_lb=2: per-sample.
    dev_lt: build the decay masks L (and C0) on device via ACT exp from a
    delta matrix instead of DMAing ~2.9 MB of host-precomputed masks."""
    nc = bacc.Bacc(None, target_bir_lowering=False)

    # Small inputs packed into two tensors (one DMA each): per-DMA issue
    # overhead (~0.65us) otherwise idles the DMA engines early on.
    pack0 = nc.dram_tensor("pack0", [CH, PACK0_W], F32R, kind="ExternalInput")
    packr = nc.dram_tensor("packr", [CH, PACKR_W], F32R, kind="ExternalInput")
    packf = nc.dram_tensor("packf", [CH, PACKF_W], F32, kind="ExternalInput")
    if not dev_lt:
        lt = nc.dram_tensor("lt", [CH, n_lb * H * 2 * T], F32, kind="ExternalInput")
        c0m = nc.dram_tensor("c0m", [N, n_lb * H * T], F32, kind="ExternalInput")
    xall = nc.dram_tensor("xall", [CH, B * NK * N], F32, kind="ExternalInput")
    # W1 head-slice, host-prearranged to [m, n*256 + j] so every DMA reads
    # long contiguous runs (full DMA rate)
    w1s = nc.dram_tensor("w1s", [N, (FLATH // N) * 256], F16, kind="ExternalInput")

    wf_out = nc.dram_tensor("wf_out", [N, B * N], F32, kind="ExternalOutput")
    cls_out = nc.dram_tensor("cls_out", [B, 256], F32, kind="ExternalOutput")
    vs_out = nc.dram_tensor("vs_out", [2, P + 1, T], F32, kind="ExternalOutput")
    z_out = nc.dram_tensor("z_out", [2, N, T], F32, kind="ExternalOutput")

    with tile.TileContext(nc) as tc:
        with (
            tc.tile_pool(name="consts", bufs=1) as consts,
            tc.tile_pool(name="work", bufs=2) as work,
            tc.tile_pool(name="spool", bufs=3) as spool,
            tc.tile_pool(name="ypool", bufs=1) as ypool,
            tc.tile_pool(name="w1pool", bufs=1) as w1pool,
            tc.tile_pool(name="psG", bufs=1, space="PSUM") as psG,
            tc.tile_pool(name="psY", bufs=2, space="PSUM") as psY,
            tc.tile_pool(name="psV", bufs=1, space="PSUM") as psV,
            tc.tile_pool(name="psZ", bufs=1, space="PSUM") as psZ,
            tc.tile_pool(name="psWF", bufs=2, space="PSUM") as psWF,
            tc.tile_pool(name="psCLS", bufs=1, space="PSUM") as psCLS,
        ):
            # ---- load constants (ordered by consumer urgency; SP queue is
            # in-order, so nothing compute-dependent may sit between these) ----
            cts = {}
            pack0_t = consts.tile([CH, PACK0_W], F32R, name="pack0_t")
            nc.sync.dma_start(out=pack0_t[:], in_=pack0[:])
            packr_t = consts.tile([CH, PACKR_W], F32R, name="packr_t")
            nc.sync.dma_start(out=packr_t[:], in_=packr[:])
            packf_t = consts.tile([CH, PACKF_W], F32, name="packf_t")
            nc.sync.dma_start(out=packf_t[:], in_=packf[:])
            for name, (rows, off, wid) in PACK0_SECT.items():
                ap = pack0_t[0:rows, off:off + wid]
                cts[name] = ap.bitcast(F32) if name in ("dmat", "lnab") else ap
            for name, (rows, off, wid) in PACKR_SECT.items():
                cts[name] = packr_t[0:rows, off:off + wid]
            for name, (rows, off, wid) in PACKF_SECT.items():
                cts[name] = packf_t[0:rows, off:off + wid]
            if not dev_lt:
                for name, hdl in [("lt", lt), ("c0m", c0m)]:
                    t = consts.tile(list(hdl.shape), F32, name=f"c_{name}")
                    nc.sync.dma_start(out=t[:], in_=hdl[:])
                    cts[name] = t
            xall_t = consts.tile([CH, B * NK * N], F32, name="xall_t")
            nc.sync.dma_start(out=xall_t[:], in_=xall[:])
            cts["xall"] = xall_t[:]
            # preload the whole W1 slice (7 groups, no slot reuse) so the SP
            # queue never stalls behind a waiting DMA
            w1tiles = []
            for g in range(NGRP):
                w1t = w1pool.tile([N, GN * 256], F16, name=f"w1t{g}", tag=f"w1t{g}")
                nc.sync.dma_start(out=w1t[:],
                                  in_=w1s[:, g * GN * 256:(g + 1) * GN * 256])
                w1tiles.append(w1t)
            wfT_r = consts.tile([N, B * N], F16, name="wfT_r")    # [m, n*16+b]
            wfT_c = consts.tile([N, B * N], F32, name="wfT_c")    # [m, b*105+n]

            # Gram matmuls first: they only need xbT (arrives first) and
            # unblock the whole part-1 producer pipeline.
            part1 = {}
            emitted_g = []

            def emit_g(i):
                g_ps = psG.tile([CH, 2 * T], F32, name="g_ps")
                for k in range(NK):
                    nc.tensor.matmul(
                        g_ps[:, k * T:(k + 1) * T],
                        cts["xbT"][:, i * T + k * CH: i * T + (k + 1) * CH],
                        cts["xbT"][:, i * T:(i + 1) * T],
                        start=True, stop=True)
                g_sb = work.tile([CH, 2 * T], F32, name="g_sb")
                nc.scalar.copy(out=g_sb[:], in_=g_ps[:])
                part1[i] = dict(g_sb=g_sb)

            emit_g(0)
            emit_g(1)

            # ---- part 2 (emitted in chunks interleaved with part 1):
            # w_final for all 16 samples for this core's head. Elementwise on
            # DVE; wfT copies on the otherwise-idle GpSimd engine.
            def emit_p2(b):
                xck = work.tile([CH, NK * N], F32, name="xck")
                for k in range(NK):
                    nc.vector.tensor_scalar_mul(
                        xck[:, k * N:(k + 1) * N],
                        cts["xall"][:, (b * NK + k) * N:(b * NK + k + 1) * N],
                        cts["dsq"][:, b * NK + k: b * NK + k + 1])
                adjs = work.tile([N, N], F32, name="adjs")
                nc.vector.tensor_scalar_mul(
                    adjs[:], cts["adjTg"][:, b * N:(b + 1) * N],
                    cts["ptv"][:, b:b + 1])
                wf_ps = psWF.tile([N, N], F32, name="wf_ps")
                for k in range(NK):
                    nc.tensor.matmul(
                        wf_ps[:], xck[:, k * N:(k + 1) * N],
                        xck[:, k * N:(k + 1) * N], start=(k == 0), stop=False)
                nc.tensor.matmul(
                    wf_ps[:], cts["ident"][:], adjs[:], start=False, stop=True)
                nc.scalar.copy(out=wfT_r[:, b:B * N:B], in_=wf_ps[:])
                nc.vector.tensor_copy(out=wfT_c[:, b * N:(b + 1) * N], in_=wf_ps[:])

            if dev_lt:
                # L_h[s, k*256+t] = eta*alpha^(t-1-s) masked: exp(lnA*dmat+lnE);
                # dmat holds +1e4 where t<=s so exp underflows to exactly 0.
                # Emitted after part 2 so these ACT ops don't block its copies.
                lm_tiles, c0_tiles = [], []
                trow = consts.tile([N, T], F32, name="trow")
                nc.gpsimd.iota(trow[:], pattern=[[1, T]], base=0,
                               channel_multiplier=0,
                               allow_small_or_imprecise_dtypes=True)
                for h in range(H):
                    lmt = consts.tile([CH, 2 * T], F32, name=f"lm{h}")
                    nc.scalar.activation(
                        out=lmt[:], in_=cts["dmat"][:],
                        func=mybir.ActivationFunctionType.Exp,
                        bias=cts["lnab"][:, 2 * h + 1:2 * h + 2],
                        scale=cts["lnab"][:, 2 * h:2 * h + 1])
                    lm_tiles.append(lmt)
                    c0t = consts.tile([N, T], F32, name=f"c0_{h}")
                    nc.scalar.activation(
                        out=c0t[:], in_=trow[:],
                        func=mybir.ActivationFunctionType.Exp,
                        bias=0.0,
                        scale=cts["lnab"][0:N, 2 * h:2 * h + 1])
                    c0_tiles.append(c0t)

            # ---- interleaved emission: cls groups + part-1 head iterations ----
            cls_ps = psCLS.tile([B, 256], F32, name="cls_ps")

            def emit_cls_group(g):
                w1t = w1tiles[g]
                for ii in range(GN):
                    n = g * GN + ii
                    nc.tensor.matmul(
                        cls_ps[:], wfT_r[:, n * B:(n + 1) * B],
                        w1t[:, ii * 256:(ii + 1) * 256],
                        start=(n == 0), stop=(n == FLATH // N - 1))

            def emit_head(i, h):
                st = part1[i]
                if dev_lt:
                    lt_ap = lm_tiles[h][:]
                    c0_ap = c0_tiles[h][:]
                else:
                    lo = ((i * H + h) if n_lb == 2 else h) * 2 * T
                    lt_ap = cts["lt"][:, lo:lo + 2 * T]
                    co = ((i * H + h) if n_lb == 2 else h) * T
                    c0_ap = cts["c0m"][:, co:co + T]
                s_sb = spool.tile([CH, 2 * T], F32R, name=f"s_sb{i}_{h}",
                                  tag=f"s_sb{i}_{h}", bufs=1)
                nc.vector.tensor_mul(out=s_sb[:], in0=lt_ap, in1=st["g_sb"][:])
                xh = spool.tile([N, T], F32R, name="xh")
                nc.vector.tensor_mul(
                    out=xh[:], in0=cts["xbT"][:, i * T:(i + 1) * T], in1=c0_ap)
                y_ps = psY.tile([N, T], F32, name="y_ps")
                for k in range(NK):
                    nc.tensor.matmul(
                        y_ps[:],
                        cts["xp"][:, (i * NK + k) * N:(i * NK + k + 1) * N],
                        s_sb[:, k * T:(k + 1) * T],
                        start=(k == 0), stop=False)
                nc.tensor.matmul(
                    y_ps[:], cts["adjTp"][:, i * N:(i + 1) * N], xh[:],
                    start=False, stop=True)
                y_sb = ypool.tile([N, T], F32R, name=f"y_sb{i}_{h}")
                nc.scalar.copy(out=y_sb[:], in_=y_ps[:])
                st[h] = y_sb

            def emit_vz(i):
                # batched V/Z accumulation: avoids the per-head PE<->ACT
                # round-trip latency in the chain
                st = part1[i]
                st["v_ps"] = psV.tile([P + 1, T], F32, name="v_ps", tag="v_ps")
                st["z_ps"] = psZ.tile([N, T], F32, name="z_ps", tag="z_ps")
                for h in range(H):
                    nc.tensor.matmul(
                        st["v_ps"][:], cts["wvu"][:, h * (P + 1):(h + 1) * (P + 1)],
                        st[h][:], start=(h == 0), stop=(h == H - 1))
                for h in range(H):
                    nc.tensor.matmul(
                        st["z_ps"][:], cts["wtrajT"][:], st[h][:],
                        start=(h == 0), stop=(h == H - 1))

            def emit_finish(i):
                st = part1[i]
                vs_sb = work.tile([P + 1, T], F32, name="vs_sb")
                nc.scalar.copy(out=vs_sb[:], in_=st["v_ps"][:])
                nc.sync.dma_start(out=vs_out[i], in_=vs_sb[:])
                z_sb = work.tile([N, T], F32, name="z_sb")
                nc.vector.tensor_copy(out=z_sb[:], in_=st["z_ps"][:])
                nc.sync.dma_start(out=z_out[i], in_=z_sb[:])

            # schedule: part-2 chunks and cls groups interleaved with
            # part-1 head iterations (engines execute in emission order)
            plan = [
                ("P2", range(0, 16)),
                ("CLS", 0), ("HD", (0, 0)), ("HD", (0, 1)), ("HD", (0, 2)),
                ("CLS", 1), ("HD", (0, 3)), ("HD", (0, 4)), ("HD", (0, 5)),
                ("CLS", 2), ("HD", (0, 6)), ("HD", (0, 7)), ("HD", (1, 0)),
                ("CLS", 3), ("VZ", 0), ("FIN", 0), ("HD", (1, 1)),
                ("CLS", 4), ("HD", (1, 2)), ("HD", (1, 3)), ("HD", (1, 4)),
                ("CLS", 5), ("HD", (1, 5)), ("HD", (1, 6)), ("HD", (1, 7)),
                ("CLS", 6), ("VZ", 1), ("FIN", 1),
            ]
            for kind, arg in plan:
                if kind == "P2":
                    for b_ in arg:
                        emit_p2(b_)
                elif kind == "HD":
                    emit_head(*arg)
                elif kind == "CLS":
                    emit_cls_group(arg)
                elif kind == "VZ":
                    emit_vz(arg)
                else:
                    emit_finish(arg)
            nc.sync.dma_start(out=wf_out[:], in_=wfT_c[:])
            cls_sb = work.tile([B, 256], F32, name="cls_sb")
            nc.scalar.copy(out=cls_sb[:], in_=cls_ps[:])
            nc.sync.dma_start(out=cls_out[:], in_=cls_sb[:])

    nc.finalize()
    return nc


def _coeffs(mask, alpha, eta):
    """Mask-general scan coefficients in float64 log space.
    C0[b,h,t] = prod_{r<t} a_hat (t=0..T); LT[b,h,s,t] = L[b,h,t,s];
    d[b,h,s] = e_s * prod_{r=s+1..T-1} a_hat."""
    m = mask.astype(np.float64)
    al = np.asarray(alpha, np.float64)
    et = np.asarray(eta, np.float64)
    a_hat = m[:, None, :] * al[None, :, None] + (1.0 - m[:, None, :])
    e_hat = m[:, None, :] * et[None, :, None]
    logc = np.concatenate(
        [np.zeros((B, H, 1)), np.cumsum(np.log(a_hat), axis=2)], axis=2)
    C0 = np.exp(logc)
    t_idx = np.arange(T)
    valid = (t_idx[None, :] > t_idx[:, None])          # [s,t]: s < t
    dlog = logc[:, :, None, 0:T] - logc[:, :, 1:T + 1, None]   # [b,h,s,t]
    with np.errstate(over="ignore"):
        LT = np.where(valid[None, None],
                      np.exp(np.where(valid[None, None], dlog, -np.inf))
                      * e_hat[:, :, :, None], 0.0)
    d = e_hat * np.exp(logc[:, :, T:T + 1] - logc[:, :, 1:T + 1])
    return C0, LT, d


def kernel(x, adj, mask, alpha, eta, query, Wk, bk, Wv, bv, Wtraj, btraj,
           W1, b1, W2, b2):
    global last_exec_time_ns, last_results
    f = np.float32
    x = np.ascontiguousarray(np.asarray(x, f))
    adj = np.ascontiguousarray(np.asarray(adj, f))
    mask = np.asarray(mask)
    alpha = np.asarray(alpha, f); eta = np.asarray(eta, f)
    query = np.asarray(query, f)
    Wk = np.asarray(Wk, f); bk = np.asarray(bk, f)
    Wv = np.asarray(Wv, f); bv = np.asarray(bv, f)
    Wtraj = np.asarray(Wtraj, f); btraj = np.asarray(btraj, f)
    W1 = np.asarray(W1, f); b1 = np.asarray(b1, f)
    W2 = np.asarray(W2, f); b2 = np.asarray(b2, f)

    uniform = bool(np.all(mask == mask[0:1]))
    dev_lt = uniform and bool(np.all(alpha >= 0) and np.all(alpha <= 0.99))
    n_lb = 1 if uniform else 2
    key = (n_lb, dev_lt)
    if key not in _progs:
        _progs[key] = _build(n_lb, dev_lt)
    nc = _progs[key]

    C0, LT, d = _coeffs(mask, alpha, eta)
    C0f = C0.astype(f)
    dsq_all = np.sqrt(d).astype(f)                      # (B,H,T)
    q = query[0, 0]
    u = (Wk @ q).astype(f)                              # (840,)
    wvu_h = np.empty((H, N, P + 1), f)
    wvu_h[:, :, :P] = Wv.reshape(H, N, P)
    wvu_h[:, :, P] = u.reshape(H, N)
    wvu_arr = np.ascontiguousarray(wvu_h.transpose(1, 0, 2).reshape(N, H * (P + 1)))
    wtrajT_arr = np.ascontiguousarray((Wtraj / float(H)).T)
    xall_arr = np.ascontiguousarray(
        x.reshape(B, NK, CH, N).transpose(2, 0, 1, 3).reshape(CH, B * NK * N))
    adjTg_arr = np.ascontiguousarray(adj.transpose(2, 0, 1).reshape(N, B * N))
    ident_arr = np.eye(N, dtype=f)

    if dev_lt:
        # device builds L/C0 via exp(lnA*delta + lnE); +1e4 in the masked
        # region underflows exp to exactly 0
        s_idx = np.arange(T)[:, None]
        t_idx = np.arange(T)[None, :]
        dm = np.where(t_idx > s_idx, (t_idx - 1 - s_idx).astype(np.float64), 1e4)
        dmat_arr = np.ascontiguousarray(
            dm.astype(f).reshape(NK, CH, T).transpose(1, 0, 2).reshape(CH, 2 * T))
        lnab_arr = np.empty((CH, 2 * H), f)
        lnab_arr[:, 0::2] = np.log(np.maximum(alpha, 1e-30))[None, :]
        lnab_arr[:, 1::2] = np.log(np.maximum(eta, 1e-38))[None, :]
    elif uniform:
        lt_shared = np.ascontiguousarray(
            LT[0].astype(f).reshape(H, NK, CH, T).transpose(2, 0, 1, 3)
            .reshape(CH, H * 2 * T))
        c0_shared = np.ascontiguousarray(np.broadcast_to(
            C0f[0, :, :T].reshape(1, H * T), (N, H * T)))

    # W1 head-blocks rearranged to [h][m, n*256+j], cast fp16 (one 90MB pass)
    W1res = np.ascontiguousarray(
        W1[:H * FLATH].reshape(H, N, N, 256).transpose(0, 2, 1, 3)
        .reshape(H, N, (FLATH // N) * 256).astype(np.float16))

    def _put(dst, sect, name, arr):
        rows, off, wid = sect[name]
        assert arr.shape == (rows, wid), (name, arr.shape, (rows, wid))
        dst[:rows, off:off + wid] = arr

    # shared sections of the packs
    pack0_tpl = np.zeros((CH, PACK0_W), f)
    if dev_lt:
        _put(pack0_tpl, PACK0_SECT, "dmat", dmat_arr)
        _put(pack0_tpl, PACK0_SECT, "lnab", lnab_arr)
    packr_tpl = np.zeros((CH, PACKR_W), f)
    _put(packr_tpl, PACKR_SECT, "wvu", wvu_arr)
    _put(packr_tpl, PACKR_SECT, "wtrajT", wtrajT_arr)
    packf_tpl = np.zeros((CH, PACKF_W), f)
    _put(packf_tpl, PACKF_SECT, "ident", ident_arr)
    _put(packf_tpl, PACKF_SECT, "adjTg", adjTg_arr)

    in_maps = []
    for qc in range(NC):
        pair = slice(2 * qc, 2 * qc + 2)
        xpair = x[pair]                                  # (2,T,N)
        pack0_a = pack0_tpl.copy()
        _put(pack0_a, PACK0_SECT, "xbT",
             xpair.transpose(0, 2, 1).transpose(1, 0, 2).reshape(N, 2 * T))
        packr_a = packr_tpl.copy()
        _put(packr_a, PACKR_SECT, "xp", np.ascontiguousarray(
            xpair.reshape(2, NK, CH, N).transpose(2, 0, 1, 3).reshape(CH, 2 * NK * N)))
        _put(packr_a, PACKR_SECT, "adjTp",
             adj[pair].transpose(2, 0, 1).reshape(N, 2 * N))
        packf_a = packf_tpl.copy()
        _put(packf_a, PACKF_SECT, "dsq",
             dsq_all[:, qc, :].reshape(B, NK, CH).transpose(2, 0, 1).reshape(CH, B * NK))
        _put(packf_a, PACKF_SECT, "ptv",
             np.broadcast_to(C0f[:, qc, T].reshape(1, B), (N, B)))
        im = {"pack0": pack0_a, "packr": packr_a, "packf": packf_a,
              "xall": xall_arr, "w1s": W1res[qc]}
        if not dev_lt:
            if uniform:
                im["lt"] = lt_shared
                im["c0m"] = c0_shared
            else:
                im["lt"] = np.ascontiguousarray(
                    LT[pair].astype(f).reshape(2, H, NK, CH, T)
                    .transpose(3, 0, 1, 2, 4).reshape(CH, 2 * H * 2 * T))
                im["c0m"] = np.ascontiguousarray(np.broadcast_to(
                    C0f[pair, :, :T].reshape(1, 2 * H * T), (N, 2 * H * T)))
        in_maps.append(im)

    trace = os.environ.get("BDH_TRACE", "") == "1"
    res = bass_utils.run_bass_kernel_spmd(
        nc, in_maps, list(range(NC)), trace=trace)
    last_exec_time_ns = res.exec_time_ns
    last_results = res

    # ---- host assembly ----
    w_final = np.empty((B, H, N, N), f)
    scores_raw = np.empty((B, T), f)
    VT = np.empty((B, P, T), f)
    x_next = np.empty((B, T, N), f)
    cls_sum = np.zeros((B, 256), np.float64)
    for qc in range(NC):
        r = res.results[qc]
        w_final[:, qc] = r["wf_out"].reshape(N, B, N).transpose(1, 2, 0)
        cls_sum += r["cls_out"].astype(np.float64)
        for i in range(2):
            b = 2 * qc + i
            scores_raw[b] = r["vs_out"][i, P]
            VT[b] = r["vs_out"][i, :P]
            x_next[b] = r["z_out"][i].T + btraj

    scale = 1.0 / np.sqrt(np.float32(P))
    scores = (scores_raw + np.float32(bk @ q)) * scale
    scores = np.where(mask == 0, f(-1e9), scores).astype(f)
    ex = np.exp((scores - scores.max(axis=1, keepdims=True)).astype(np.float64))
    attn = ex / ex.sum(axis=1, keepdims=True)
    context = np.einsum("bt,bpt->bp", attn, VT.astype(np.float64)) + bv

    W1c = W1[H * FLATH:, :].astype(np.float64)           # context rows of W1
    h1 = cls_sum + context @ W1c + b1
    h1 = np.maximum(h1, 0.0)
    logits = (h1 @ W2.astype(np.float64) + b2).astype(f)

    return logits, x_next, w_final
